# revision 4
# baseline (speedup 1.0000x reference)
"""Trainium2 Bass kernel for nn_NeuronPool (retrieval_knn).

Computes, for x:[B,S,D], neurons:[N,D], W_q:[D,D], b_q:[D], k=8:
    q       = x @ W_q.T + b_q
    scores  = q @ neurons.T
    topv,ti = top_k(scores, 8)
    w       = softmax(topv)
    out     = sum_k w_k * neurons[ti_k]
returns (out:[B,S,D] f32, ti:[B,S,8] i32, w:[B,S,8] f32)

Sharding: data-parallel over the B*S=16384 tokens -> 2048 tokens/core on 8
cores; neurons / W_q replicated.  All matmuls in fp32 on the PE (exact-enough
to reproduce the fp32 reference's top-k indices up to the irreducible
near-tie noise floor).  Top-k via the hardware MAX8/MAX_INDEX instructions on
per-512..256-column score chunks held in PSUM->SBUF, combined with a
position-based one-hot extraction that matches jax.lax.top_k tie-breaking.
The k selected neuron rows are fetched with dma_gather and combined with
ACT-scaled copies + DVE adds.
"""

import numpy as np
from contextlib import ExitStack

import concourse.bass as bass
import concourse.mybir as mybir
import concourse.tile as tile
from concourse import bacc
from concourse.bass_utils import run_bass_kernel_spmd

# problem shape (hardcoded per harness contract)
B, S, D, N, K = 4, 4096, 2048, 8192, 8
NCORES = 8
TOK = B * S                  # 16384 total tokens
TPC = TOK // NCORES          # 2048 tokens per core
P = 128

SB_TOK = 512                 # tokens per superblock
NSB = TPC // SB_TOK          # 4 superblocks
NTT = SB_TOK // P            # 4 token-tiles per superblock
CH = 256                     # stage-2 score chunk width
NCH = N // CH                # 32 chunks
ECH = D // P                 # 16 contraction tiles (128 each)
NCAND = NCH * K              # 256 candidates per token
GROWS = 2                    # neuron rows per gather DMA

dt = mybir.dt
AF = mybir.ActivationFunctionType
OP = mybir.AluOpType


def _build_program():
    nc = bacc.Bacc("TRN2", target_bir_lowering=False, debug=False,
                   num_devices=NCORES)

    xT = nc.dram_tensor("xT", [D, TPC], dt.float32, kind="ExternalInput").ap()
    WqT = nc.dram_tensor("WqT", [D, D], dt.float32, kind="ExternalInput").ap()
    nT = nc.dram_tensor("nT", [D, N], dt.float32, kind="ExternalInput").ap()
    neur = nc.dram_tensor("neurons", [N, D], dt.float32,
                          kind="ExternalInput").ap()
    bq = nc.dram_tensor("bq", [D], dt.float32, kind="ExternalInput").ap()

    out = nc.dram_tensor("out", [TPC, D], dt.float32,
                         kind="ExternalOutput").ap()
    oidx = nc.dram_tensor("oidx", [TPC, K], dt.int32,
                          kind="ExternalOutput").ap()
    ow = nc.dram_tensor("ow", [TPC, K], dt.float32,
                        kind="ExternalOutput").ap()

    # per-(sb,tt) DRAM scratch for the int16 index round-trip
    scratch = [nc.dram_tensor(f"scr_{i}", [P, K], dt.int16).ap()
               for i in range(NSB * NTT)]

    xT_r = xT.rearrange("(do dp) t -> dp do t", dp=P)
    WqT_r = WqT.rearrange("(do dp) e -> dp do e", dp=P)
    nT_r = nT.rearrange("(eo ep) n -> ep eo n", ep=P)

    with tile.TileContext(nc) as tc, ExitStack() as ctx:
        const = ctx.enter_context(tc.tile_pool(name="const", bufs=1))
        qpool = ctx.enter_context(tc.tile_pool(name="qpool", bufs=1))
        xpool = ctx.enter_context(tc.tile_pool(name="xpool", bufs=1))
        wqpool = ctx.enter_context(tc.tile_pool(name="wqpool", bufs=2))
        ntpool = ctx.enter_context(tc.tile_pool(name="ntpool", bufs=2))
        scpool = ctx.enter_context(tc.tile_pool(name="scpool", bufs=3))
        candp = ctx.enter_context(tc.tile_pool(name="candp", bufs=1))
        smallp = ctx.enter_context(tc.tile_pool(name="smallp", bufs=2))
        gpool = ctx.enter_context(tc.tile_pool(name="gpool", bufs=2))
        accp = ctx.enter_context(tc.tile_pool(name="accp", bufs=2))
        tmpp = ctx.enter_context(tc.tile_pool(name="tmpp", bufs=2))
        psq = ctx.enter_context(tc.tile_pool(name="psq", bufs=2, space="PSUM"))
        pss = ctx.enter_context(tc.tile_pool(name="pss", bufs=4, space="PSUM"))

        # ---- one-time constants ----
        bq_sb = const.tile([P, ECH], dt.float32)
        nc.sync.dma_start(bq_sb[:], bq.rearrange("(eo ep) -> ep eo", ep=P))

        off_i = const.tile([P, NCH, K], dt.int32)
        nc.gpsimd.iota(off_i, pattern=[[CH, NCH], [0, K]], base=0,
                       channel_multiplier=0)
        off_f = const.tile([P, NCH, K], dt.float32)
        nc.vector.tensor_copy(off_f[:], off_i[:])

        pos_i = const.tile([P, NCAND], dt.int32)
        nc.gpsimd.iota(pos_i, pattern=[[1, NCAND]], base=0,
                       channel_multiplier=0)
        pos_f = const.tile([P, NCAND], dt.float32)
        nc.vector.tensor_copy(pos_f[:], pos_i[:])

        for sb in range(NSB):
            t0 = sb * SB_TOK

            # ================= stage 1: qT[e, t] for this superblock ======
            xT_sb = xpool.tile([P, ECH, SB_TOK], dt.float32)
            nc.sync.dma_start(xT_sb[:], xT_r[:, :, t0:t0 + SB_TOK])

            qT_sb = qpool.tile([P, ECH, SB_TOK], dt.float32)
            for et in range(ECH):
                wq_et = wqpool.tile([P, ECH, P], dt.float32)
                nc.sync.dma_start(wq_et[:],
                                  WqT_r[:, :, et * P:(et + 1) * P])
                ps_q = psq.tile([P, SB_TOK], dt.float32)
                for do in range(ECH):
                    nc.tensor.matmul(ps_q[:], lhsT=wq_et[:, do, :],
                                     rhs=xT_sb[:, do, :],
                                     start=(do == 0), stop=(do == ECH - 1))
                # psum -> sbuf with bias add
                nc.vector.tensor_scalar(qT_sb[:, et, :], ps_q[:],
                                        bq_sb[:, et:et + 1], None, op0=OP.add)

            # ================= stage 2: score chunks + local top-8 ========
            cand_v = candp.tile([P, NTT, NCH, K], dt.float32)
            cand_i = candp.tile([P, NTT, NCH, K], dt.uint16)
            for ci in range(NCH):
                nt_ch = ntpool.tile([P, ECH, CH], dt.float32)
                nc.sync.dma_start(nt_ch[:],
                                  nT_r[:, :, ci * CH:(ci + 1) * CH])
                for tt in range(NTT):
                    ps_s = pss.tile([P, CH], dt.float32)
                    for eo in range(ECH):
                        nc.tensor.matmul(
                            ps_s[:],
                            lhsT=qT_sb[:, eo, tt * P:(tt + 1) * P],
                            rhs=nt_ch[:, eo, :],
                            start=(eo == 0), stop=(eo == ECH - 1))
                    sc_b = scpool.tile([P, CH], dt.float32)
                    nc.scalar.copy(sc_b[:], ps_s[:])
                    nc.vector.max(out=cand_v[:, tt, ci], in_=sc_b[:])
                    nc.vector.max_index(out=cand_i[:, tt, ci],
                                        in_max=cand_v[:, tt, ci],
                                        in_values=sc_b[:])

            # ================= stage 3: combine, softmax, gather, wsum ====
            for tt in range(NTT):
                gtt = sb * NTT + tt
                rows = slice(gtt * P, (gtt + 1) * P)

                cv = cand_v[:, tt].rearrange("p a b -> p (a b)")
                top8 = smallp.tile([P, K], dt.float32)
                nc.vector.max(out=top8[:], in_=cv)
                pos8 = smallp.tile([P, K], dt.uint16)
                nc.vector.max_index(out=pos8[:], in_max=top8[:], in_values=cv)
                pos8f = smallp.tile([P, K], dt.float32)
                nc.vector.tensor_copy(pos8f[:], pos8[:])

                # candidate global index (fp32, exact)
                cidx_f = smallp.tile([P, NCH, K], dt.float32)
                nc.vector.tensor_copy(cidx_f[:], cand_i[:, tt])
                nc.vector.tensor_add(cidx_f[:], cidx_f[:], off_f[:])
                cidx_flat = cidx_f[:, :, :].rearrange("p a b -> p (a b)")

                idxf = smallp.tile([P, K], dt.float32)
                onehot = scpool.tile([P, NCAND], dt.float32, tag="onehot")
                scr = scpool.tile([P, NCAND], dt.float32, tag="ohscr")
                for j in range(K):
                    nc.vector.tensor_scalar(onehot[:], pos_f[:],
                                            pos8f[:, j:j + 1], None,
                                            op0=OP.is_equal)
                    nc.vector.tensor_mul(scr[:], onehot[:], cidx_flat)
                    nc.vector.reduce_sum(idxf[:, j:j + 1], scr[:],
                                         axis=mybir.AxisListType.X)

                idx32 = smallp.tile([P, K], dt.int32)
                nc.vector.tensor_copy(idx32[:], idxf[:])
                nc.sync.dma_start(oidx[rows, :], idx32[:])
                idx16 = smallp.tile([P, K], dt.int16)
                nc.vector.tensor_copy(idx16[:], idxf[:])
                nc.sync.dma_start(scratch[gtt][:, :], idx16[:])

                # softmax over top8 (descending, so max is col 0)
                negm = smallp.tile([P, 1], dt.float32)
                nc.vector.tensor_scalar_mul(negm[:], top8[:, 0:1], -1.0)
                e8 = smallp.tile([P, K], dt.float32)
                sume = smallp.tile([P, 1], dt.float32)
                nc.scalar.activation(e8[:], top8[:], AF.Exp, bias=negm[:],
                                     scale=1.0, accum_out=sume[:])
                rec = smallp.tile([P, 1], dt.float32)
                nc.vector.reciprocal(rec[:], sume[:])
                w8 = smallp.tile([P, K], dt.float32)
                nc.vector.tensor_scalar(w8[:], e8[:], rec[:], None,
                                        op0=OP.mult)
                nc.sync.dma_start(ow[rows, :], w8[:])

                # gather the K selected rows (GROWS rows per DMA) + wsum
                acc = accp.tile([P, D], dt.float32)
                idxs_t = smallp.tile([P, K, K], dt.int16, tag="gidx")
                # idx pattern must be replicated into each Q7 core's
                # 16-partition group
                for kg in range(8):
                    nc.sync.dma_start(
                        idxs_t[16 * kg:16 * (kg + 1), :, :],
                        scratch[gtt].rearrange("(u q) c -> q c u", q=16))
                for g in range(K // GROWS):
                    G = gpool.tile([P, GROWS, D], dt.float32)
                    nc.gpsimd.dma_gather(
                        out_ap=G[:],
                        in_ap=neur[:],
                        idxs_ap=idxs_t[:, g * GROWS:(g + 1) * GROWS, :],
                        num_idxs=P * GROWS, num_idxs_reg=P * GROWS,
                        elem_size=D)
                    for r in range(GROWS):
                        j = g * GROWS + r
                        dst = acc if j == 0 else tmpp.tile([P, D], dt.float32,
                                                           tag="wtmp")
                        nc.scalar.activation(dst[:], G[:, r], AF.Copy,
                                             bias=0.0, scale=w8[:, j:j + 1])
                        if j > 0:
                            nc.vector.tensor_add(acc[:], acc[:], dst[:])
                nc.sync.dma_start(out[rows, :], acc[:])

    nc.compile()
    return nc


_CACHED = None


def _get_program():
    global _CACHED
    if _CACHED is None:
        _CACHED = _build_program()
    return _CACHED


def _make_in_maps(x, neurons, W_q, b_q):
    x = np.ascontiguousarray(np.asarray(x, dtype=np.float32)).reshape(TOK, D)
    neurons = np.ascontiguousarray(np.asarray(neurons, dtype=np.float32))
    W_q = np.asarray(W_q, dtype=np.float32)
    b_q = np.ascontiguousarray(np.asarray(b_q, dtype=np.float32))
    WqT = np.ascontiguousarray(W_q.T)
    nT = np.ascontiguousarray(neurons.T)
    in_maps = []
    for c in range(NCORES):
        xs = x[c * TPC:(c + 1) * TPC]          # [TPC, D]
        in_maps.append({
            "xT": np.ascontiguousarray(xs.T),  # [D, TPC]
            "WqT": WqT,
            "nT": nT,
            "neurons": neurons,
            "bq": b_q,
        })
    return in_maps


def run_sharded(x, neurons, W_q, b_q, trace=False):
    """Run on the 8 NeuronCores; returns (out, idx, w) in token-major shape
    plus the BassKernelResults (for exec_time_ns when trace=True)."""
    nc = _get_program()
    in_maps = _make_in_maps(x, neurons, W_q, b_q)
    res = run_bass_kernel_spmd(nc, in_maps, list(range(NCORES)), trace=trace)
    outs = np.concatenate([res.results[c]["out"] for c in range(NCORES)], 0)
    idxs = np.concatenate([res.results[c]["oidx"] for c in range(NCORES)], 0)
    ws = np.concatenate([res.results[c]["ow"] for c in range(NCORES)], 0)
    return outs, idxs, ws, res


def kernel(x, neurons, W_q, b_q, k=K, **_ignored):
    assert int(k) == K
    outs, idxs, ws, _ = run_sharded(x, neurons, W_q, b_q, trace=False)
    return (outs.reshape(B, S, D).astype(np.float32),
            idxs.reshape(B, S, K).astype(np.int32),
            ws.reshape(B, S, K).astype(np.float32))
